# revision 52
# baseline (speedup 1.0000x reference)
"""Trainium2 Bass kernel for nn_MultiHeadHighLevelAllocator.

Math (reference):
    uav_embed = MLP_u(uav_feat)                     # (U=256, E=128)
    task_embed = MLP_t(task_feat)                   # (T=512, E=128)
    uq[h,u,:]  = uav_embed[u] + head_queries[h]     # (H=4, U, E)
    a[hu,k]    = uq[hu] @ Wu.T + fb0                # Wu = fw0[:, :E]
    b[t,k]     = task_embed[t] @ Wt.T               # Wt = fw0[:, E:]
    logits[hu,t] = sum_k fw1[k] * relu(a[hu,k] + b[t,k]) + fb1

Strategy (8 cores, shard T -> 64 t's per core, full HU on every core):
    - Prep matmuls on PE in feature-on-partition layout (host pre-transposes
      inputs); a = uqT@Wu + fb0 evicted to fp16 a16[k, hu] (2 k-tiles of
      (128, 1024)); b[k, t_local] f32; uq/teT/b evictions on DVE (idle at
      ramp), a16 on ACT.
    - Fused bias+ReLU per (t, ktile) unit over the (128k, 1024hu) plane:
      DVE tensor_scalar add+max in 4x_2p mode (HW ~0.35-0.39us/unit) or ACT
      relu-with-bias (~1.14us/unit); split via CFG["x"].
    - Contraction with fw1 on PE fp16: lhsT = fw1 k-slice (128,1), rhs = R
      (128,512) halves, M=1 outputs col-tiled to PSUM partitions
      {0,32,64,96} (4 bands run concurrently, HW ~86ns/matmul), accumulated
      over k-tiles. 1-round (128, 1024) psum groups x3 bufs (6 banks),
      prep psum in its own 2-bank pool.
    - ACT evicts each group (+fb1, identity-bias); one strided DMA per
      group writes DRAM [16, 4, HU] (t = 4g + j). DMA cannot read PSUM and
      matmul psum bases must be 32-aligned, so denser layouts are out.
    - Software pipelining: For_i carries an all-engine barrier per
      iteration (ramp+tail serialize), so the timed loop unrolls
      BENCH_UNROLL full kernels per iteration with parity-alternated
      uqT/a16/b buffers; kernel u+1's prep is emitted mid-fusion of kernel
      u as 13 stages on a tuned per-stage schedule (CFG[prep_pace]: the
      ACT-dependent encoder stages spaced ~1 group apart so units slot
      between the PE<->ACT ping-pong hops, a16 stages at groups 10-11).
      Bunched emission head-of-line blocks ACT on the chain latency
      (measured +3.3us); uniform 1/group pacing lands a16 too late (+5us).
      Measured on HW: DVE is the binding engine (~108 units x ~0.41us incl
      per-instr overhead); x=20 with delayed-1-group evictions balances
      ACT at ~43us busy.
    - Input weights consolidated into one (128, 1036) packed DMA + 5 small
      loads spread over the SP/ACT DGE queues (SP issue is ~1us/DMA).

Output per core: [16, 4, HU] f32, t = 4g + j; host reassembles (H,U,T).
"""

import contextlib

import numpy as np

import concourse.bacc as bacc
import concourse.mybir as mybir
from concourse.tile import TileContext
from concourse.bass_utils import run_bass_kernel_spmd

U, T, H = 256, 512, 4
UAV_DIM, TASK_DIM, E, HID = 64, 32, 128, 256
HU = H * U                      # 1024
NCORES = 8
TL = T // NCORES                # 64 t's per core
NKT = HID // 128                # 2 k-tiles
NROUNDS = TL // 4               # 16 rounds of 4 t's

f32 = mybir.dt.float32
f16 = mybir.dt.float16
f32r = mybir.dt.float32r
AF = mybir.ActivationFunctionType
ALU = mybir.AluOpType
ET = mybir.EngineType

BENCH_UNROLL = 4

# Tunables; _get_nc caches on their values.
#   x: total ACT units of 128 (16 -> (0,0) every round; +1 per extra / 16)
#   rpool: R-tile pool depth; prep_at: fusion group after which the next
#   kernel's prep is emitted; ldq: spread loads over SP+ACT DGE queues
CFG = {"x": 20, "rpool": 32, "opool": 3, "prep_dve": 1, "ldq": 1,
       "uq_eng": "dve", "b_eng": "dve",
       "prep_at": 4, "prep_at2": 11,
       # stages: s0 s1 s2 s3 s4 uq0 uq1 uq2 uq3 teT b0 b1 a00 a01 a10 a11
       "prep_pace": (3, 4, 4, 5, 5, 6, 6, 6, 6, 6, 6, 6, 10, 10, 11, 11)}


def _act_units_for_round(r):
    x = CFG["x"]
    base = {(0, 0)} if x >= 16 else ({(0, 0)} if r % 2 == 0 else set())
    extra = x - 16
    rp = (r + CFG.get("xphase", 0)) % 16
    if extra > 0 and (rp * extra) // 16 != ((rp + 1) * extra) // 16:
        base = base | {(0, 1)}
    return base


# Packed weight block: name -> column width in the (128, NW) f32 tensor.
PACK = [
    ("uw1T", 128), ("tw1T", 128),
    ("ub0c", 1), ("ub1c", 1), ("tb0c", 1), ("tb1c", 1), ("tb2c", 1),
    ("uw2T", 128), ("tw2T", 128), ("hq2T", 4),
    ("WuT", 256), ("WtT", 256), ("fb0c", 2), ("fb1s", 1),
]
PACK_OFF = {}
_off = 0
for _n, _w in PACK:
    PACK_OFF[_n] = (_off, _w)
    _off += _w
NW = _off

IN_SPECS = [
    ("uavT", (UAV_DIM, U), f32),
    ("uw0T", (UAV_DIM, 128), f32),
    ("taskT", (TASK_DIM, TL), f32),
    ("tw0T", (TASK_DIM, 128), f32),
    ("wpack", (128, NW), f32),
    ("fw1c", (128, NKT), f16),
]


def _emit_loads(nc, d, singles):
    s = {}
    qs = ([nc.sync, nc.scalar] if CFG["ldq"] else [nc.sync])
    for i, (name, shape, dt_) in enumerate(IN_SPECS):
        s[name] = singles.tile(list(shape), dt_, name=name, tag=name)
        qs[i % len(qs)].dma_start(out=s[name], in_=d[name][:])
    wp = s["wpack"]
    for name, (off, w) in PACK_OFF.items():
        s[name] = wp[:, off : off + w]
    return s


def _alloc_ab(singles, parity):
    """Per-parity prep outputs (double buffered across pipelined kernels)."""
    return {
        "uqT": singles.tile([E, HU], f32, name=f"uqT{parity}",
                            tag=f"uqT{parity}"),
        "a16": [singles.tile([128, HU], f16, name=f"a16_{kt}_{parity}",
                             tag=f"a16_{kt}_{parity}") for kt in range(NKT)],
        "b": [singles.tile([128, TL], f32, name=f"b{kt}_{parity}",
                           tag=f"b{kt}_{parity}") for kt in range(NKT)],
    }


_uid = [0]


def _prep_stages(nc, s, prep, ppsum, ab):
    """Prep broken into small stages for spread emission mid-fusion.

    Returns a list of closures; emitting them in order (possibly
    interleaved with fusion groups) reproduces _emit_prep exactly.
    """
    u = _uid[0]
    _uid[0] += 1
    uqT_s, a16_s, b_s = ab["uqT"], ab["a16"], ab["b"]
    st = []
    box = {}

    def s0():
        box["pe1"] = ppsum.tile([128, U], f32, tag="ps_p", name=f"pe1_{u}")
        nc.tensor.matmul(box["pe1"], s["uw0T"], s["uavT"],
                         start=True, stop=True)
        box["pt1"] = ppsum.tile([128, TL], f32, tag="ps_p", name=f"pt1_{u}")
        nc.tensor.matmul(box["pt1"], s["tw0T"], s["taskT"],
                         start=True, stop=True)

    def s1():
        box["h1"] = prep.tile([128, U], f32, tag="pr", name=f"h1_{u}")
        nc.scalar.activation(box["h1"], box["pe1"], AF.Relu,
                             bias=s["ub0c"][:, 0:1])
        box["s1"] = prep.tile([128, TL], f32, tag="pr", name=f"s1_{u}")
        nc.scalar.activation(box["s1"], box["pt1"], AF.Relu,
                             bias=s["tb0c"][:, 0:1])

    def s2():
        box["pe2"] = ppsum.tile([128, U], f32, tag="ps_p", name=f"pe2_{u}")
        nc.tensor.matmul(box["pe2"], s["uw1T"], box["h1"],
                         start=True, stop=True)
        box["pt2"] = ppsum.tile([128, TL], f32, tag="ps_p", name=f"pt2_{u}")
        nc.tensor.matmul(box["pt2"], s["tw1T"], box["s1"],
                         start=True, stop=True)

    def s3():
        box["h2"] = prep.tile([128, U], f32, tag="pr", name=f"h2_{u}")
        nc.scalar.activation(box["h2"], box["pe2"], AF.Relu,
                             bias=s["ub1c"][:, 0:1])
        box["s2"] = prep.tile([128, TL], f32, tag="pr", name=f"s2_{u}")
        nc.scalar.activation(box["s2"], box["pt2"], AF.Relu,
                             bias=s["tb1c"][:, 0:1])

    def s4():
        box["pe3"] = ppsum.tile([E, U], f32, tag="ps_p", name=f"pe3_{u}")
        nc.tensor.matmul(box["pe3"], s["uw2T"], box["h2"],
                         start=True, stop=True)
        box["pt3"] = ppsum.tile([E, TL], f32, tag="ps_p", name=f"pt3_{u}")
        nc.tensor.matmul(box["pt3"], s["tw2T"], box["s2"],
                         start=True, stop=True)

    def mk_uq(h):
        def f():
            if CFG["uq_eng"] == "dve":
                nc.vector.tensor_scalar(
                    out=uqT_s[:, h * U : (h + 1) * U], in0=box["pe3"],
                    scalar1=s["hq2T"][:, h : h + 1], scalar2=None,
                    op0=ALU.add)
            else:
                nc.scalar.activation(
                    uqT_s[:, h * U : (h + 1) * U], box["pe3"], AF.Identity,
                    bias=s["hq2T"][:, h : h + 1])
        return f

    def s6():
        teT = prep.tile([E, TL], f32, tag="pr", name=f"teT_{u}")
        if CFG["prep_dve"]:
            nc.vector.tensor_scalar(out=teT, in0=box["pt3"],
                                    scalar1=s["tb2c"][:, 0:1], scalar2=None,
                                    op0=ALU.add)
        else:
            nc.scalar.activation(teT, box["pt3"], AF.Identity,
                                 bias=s["tb2c"][:, 0:1])
        box["teT"] = teT

    def mk_b(kt):
        def f():
            pb = ppsum.tile([128, TL], f32, tag="ps_p", name=f"pb{kt}_{u}")
            nc.tensor.matmul(pb, s["WtT"][:, kt * 128 : (kt + 1) * 128],
                             box["teT"], start=True, stop=True)
            if CFG["b_eng"] == "act":
                nc.scalar.copy(out=b_s[kt], in_=pb)
            else:
                nc.vector.tensor_copy(out=b_s[kt], in_=pb)
        return f

    def mk_a16(kt, half):
        def f():
            pa = ppsum.tile([128, 512], f32, tag="ps_p",
                            name=f"pa{kt}{half}_{u}")
            nc.tensor.matmul(
                pa, s["WuT"][:, kt * 128 : (kt + 1) * 128],
                uqT_s[:, half * 512 : (half + 1) * 512],
                start=True, stop=True,
            )
            nc.scalar.activation(
                a16_s[kt][:, half * 512 : (half + 1) * 512], pa,
                AF.Identity, bias=s["fb0c"][:, kt : kt + 1],
            )
        return f

    st = [s0, s1, s2, s3, s4,
          mk_uq(0), mk_uq(1), mk_uq(2), mk_uq(3), s6,
          mk_b(0), mk_b(1),
          mk_a16(0, 0), mk_a16(0, 1), mk_a16(1, 0), mk_a16(1, 1)]
    return st


def _emit_prep(nc, s, prep, ppsum, ab):
    for f in _prep_stages(nc, s, prep, ppsum, ab):
        f()


def _emit_fusion(nc, d, s, rpool, opool, fpsum, ab, mid_hook=None):
    """16 1-round groups: units -> contraction -> evict(+fb1) -> DMA."""
    u = _uid[0]
    _uid[0] += 1
    a16_s, b_s = ab["a16"], ab["b"]
    pending = []

    def evict(g, ps_g):
        o_st = opool.tile([128, HU], f32, tag="o", name=f"o{u}_{g}")
        nc.scalar.activation(o_st, ps_g, AF.Identity, bias=s["fb1s"][:, 0:1])
        osrc = o_st.rearrange("(j i) n -> j i n", j=4)
        nc.sync.dma_start(out=d["out4"][g], in_=osrc[:, 0, :])

    for g in range(NROUNDS):
        ps_g = fpsum.tile([128, HU], f32, tag="ps_g", name=f"ps_{u}_{g}")
        act_units = _act_units_for_round(g)
        rt = {}
        for kt in range(NKT):
            for j in range(4):
                t = 4 * g + j
                Rt = rpool.tile([128, HU], f16, tag="R",
                                name=f"R{u}_{g}_{j}_{kt}")
                bias_ap = b_s[kt][:, t : t + 1]
                if (j, kt) in act_units:
                    nc.scalar.activation(Rt, a16_s[kt], AF.Relu, bias=bias_ap)
                else:
                    nc.vector.tensor_scalar(
                        out=Rt, in0=a16_s[kt], scalar1=bias_ap,
                        scalar2=0.0, op0=ALU.add, op1=ALU.max,
                    )
                rt[(j, kt)] = Rt
        for kt in range(NKT):
            for half in range(2):
                for j in (1, 2, 3, 0):
                    nc.tensor.matmul(
                        ps_g[32 * j : 32 * j + 1,
                             half * 512 : (half + 1) * 512],
                        s["fw1c"][:, kt : kt + 1],
                        rt[(j, kt)][:, half * 512 : (half + 1) * 512],
                        start=(kt == 0), stop=(kt == NKT - 1),
                        tile_position=(0, 32 * j),
                    )
        pending.append((g, ps_g))
        if len(pending) > 1:
            evict(*pending.pop(0))
        if mid_hook is not None:
            mid_hook(g)
    while pending:
        evict(*pending.pop(0))


def _build_nc(loop=None, unroll=1):
    nc = bacc.Bacc(None, target_bir_lowering=False)
    d = {}
    for name, shape, dt_ in IN_SPECS:
        d[name] = nc.dram_tensor(name, list(shape), dt_, kind="ExternalInput")
    d["out4"] = nc.dram_tensor("out", [NROUNDS, 4, HU], f32,
                               kind="ExternalOutput")

    _uid[0] = 0
    with TileContext(nc) as tc:
        with tc.tile_pool(name="singles", bufs=1) as singles, \
             tc.tile_pool(name="prep", bufs=4) as prep, \
             tc.tile_pool(name="rpool", bufs=CFG["rpool"]) as rpool, \
             tc.tile_pool(name="opool", bufs=CFG.get("opool", 3)) as opool, \
             tc.tile_pool(name="ppsum", bufs=2, space="PSUM") as ppsum, \
             tc.tile_pool(name="fpsum", bufs=3, space="PSUM") as fpsum:
            s = _emit_loads(nc, d, singles)
            ab = [_alloc_ab(singles, p) for p in range(2)]
            _emit_prep(nc, s, prep, ppsum, ab[0])
            ctx = (tc.For_i(0, loop, 1,
                            hint_engines=(ET.PE, ET.Activation, ET.DVE))
                   if loop else contextlib.nullcontext())
            with ctx:
                for uu in range(unroll):
                    par = uu % 2
                    if loop or uu + 1 < unroll:
                        stages = _prep_stages(nc, s, prep, ppsum,
                                              ab[(uu + 1) % 2])
                        state = {"i": 0}

                        def hook(g, stages=stages, state=state):
                            # pace None: encoder stages (0..8) bunched at
                            # prep_at, a16 stages (9..12) at prep_at2.
                            # pace (pn, pd): stage i due at group
                            # prep_at + (i*pd)//pn. Remainder flushes at
                            # the last group.
                            pace = CFG["prep_pace"]

                            def due(i):
                                if isinstance(pace, (tuple, list)) \
                                        and len(pace) == 16:
                                    return pace[i]
                                if pace is None:
                                    return (CFG["prep_at"] if i < 12
                                            else CFG["prep_at2"])
                                return CFG["prep_at"] + (i * pace[1]) // pace[0]

                            while state["i"] < len(stages) and (
                                    due(state["i"]) <= g
                                    or g == NROUNDS - 1):
                                stages[state["i"]]()
                                state["i"] += 1
                    else:
                        hook = None
                    _emit_fusion(nc, d, s, rpool, opool, fpsum, ab[par],
                                 mid_hook=hook)

    nc.finalize()
    return nc


_NC_CACHE = {}


def _get_nc(loop=None, unroll=1):
    key = (loop, unroll, tuple(sorted(CFG.items())))
    if key not in _NC_CACHE:
        _NC_CACHE[key] = _build_nc(loop, unroll)
    return _NC_CACHE[key]


def _prep_inputs(inputs):
    ct = np.ascontiguousarray
    f = np.float32
    uav_feat = inputs["uav_feat"].astype(f)
    task_feat = inputs["task_feat"].astype(f)
    packed = {
        "uw1T": ct(inputs["uw1"].T.astype(f)),
        "tw1T": ct(inputs["tw1"].T.astype(f)),
        "ub0c": ct(inputs["ub0"].astype(f).reshape(128, 1)),
        "ub1c": ct(inputs["ub1"].astype(f).reshape(128, 1)),
        "tb0c": ct(inputs["tb0"].astype(f).reshape(128, 1)),
        "tb1c": ct(inputs["tb1"].astype(f).reshape(128, 1)),
        "tb2c": ct(inputs["tb2"].astype(f).reshape(128, 1)),
        "uw2T": ct(inputs["uw2"].T.astype(f)),
        "tw2T": ct(inputs["tw2"].T.astype(f)),
        "hq2T": ct((inputs["head_queries"].astype(f)
                    + inputs["ub2"].astype(f)[None, :]).T),
        "WuT": ct(inputs["fw0"][:, :E].T.astype(f)),
        "WtT": ct(inputs["fw0"][:, E:].T.astype(f)),
        "fb0c": ct(inputs["fb0"].astype(f).reshape(NKT, 128).T),
        "fb1s": ct(np.full((128, 1), float(inputs["fb1"][0]), dtype=f)),
    }
    wpack = np.empty((128, NW), dtype=f)
    for name, (off, w) in PACK_OFF.items():
        wpack[:, off : off + w] = packed[name]
    base = {
        "uavT": ct(uav_feat.T),
        "uw0T": ct(inputs["uw0"].T.astype(f)),
        "tw0T": ct(inputs["tw0"].T.astype(f)),
        "wpack": ct(wpack),
        "fw1c": ct(inputs["fw1"].reshape(NKT, 128).T.astype(np.float16)),
    }
    taskT_full = ct(task_feat.T)
    in_maps = []
    for c in range(NCORES):
        m = dict(base)
        m["taskT"] = ct(taskT_full[:, c * TL : (c + 1) * TL])
        in_maps.append(m)
    return in_maps


def run(trace=False, **inputs):
    nc = _get_nc()
    in_maps = _prep_inputs(inputs)
    res = run_bass_kernel_spmd(nc, in_maps, list(range(NCORES)), trace=trace)
    big = np.concatenate(
        [res.results[c]["out"].reshape(TL, HU) for c in range(NCORES)],
        axis=0)
    out = np.ascontiguousarray(big.T).reshape(H, U, T)
    return out, res


def kernel(**inputs):
    out, _ = run(**inputs)
    return out
